# revision 5
# baseline (speedup 1.0000x reference)
"""Trainium2 Bass kernel for the per-species (MoE-routed) atom MLP net.

Computation (see reference):
  x: [B=2048, A=64, D=384] f32, species: [A] int32 in [0, S=4)
  4-layer per-species MLP 384->128->128->64->1 with gaussian act exp(-y^2)
  between layers, then sum over atoms -> out [B].

Strategy:
  - Data-parallel over B across 8 cores (B_c = 256), no collectives.
  - Host-side: repack x into [feature-on-partition, batch-on-free] layout so
    all device DMAs are fully contiguous; group atoms by species into "units"
    of up to 2 atoms (N = 256 * n_atoms <= 512 matmul free dim, fp32).
  - exp(-(y+b)^2) is computed in ONE ScalarE op via Derivative_Erf:
    d/dx erf(x) = (2/sqrt(pi)) * exp(-x^2); the 2/sqrt(pi) factor is folded
    into the next layer's weights on the host (sqrt(pi)/2 scaling).
  - Layer 3 matmuls accumulate all units into one [1, 512] PSUM bank; the
    final fold (cols b + cols 256+b, + sum of b3 biases) happens on host.
"""

import math

import numpy as np

import concourse.bass as bass
import concourse.mybir as mybir
import concourse.tile as tile
from concourse.bass_utils import run_bass_kernel_spmd
from concourse.vector_clock import ScopedClock

AF = mybir.ActivationFunctionType
F32 = mybir.dt.float32

B, A, D, S = 2048, 64, 384, 4
H1, H2, H3 = 128, 128, 64
NCORES = 8
BC = B // NCORES  # 256 batch per core
NCH = D // 128  # 3 k-chunks for layer 0

# Set by test harness to collect a profile; kernel() stores exec_time_ns here.
PROFILE = False
LAST_EXEC_NS = None
LAST_RESULTS = None

# ---------------------------------------------------------------------------
# Walrus in this toolchain rejects >1 sync-wait per instruction ("Too many
# sync wait commands", CoreV3GenImpl setupSyncWait).  Tile's semaphore
# assignment freely attaches several waits to one instruction, so any real
# Tile kernel trips it.  Post-pass: hoist all but one wait onto injected
# NoOps on the same engine queue immediately before the instruction — the
# queue executes them in order, so the blocking semantics are identical.
# ---------------------------------------------------------------------------


def _split_multi_waits(nc):
    import bass_rust

    n_split = 0
    for blk in nc.main_func.blocks:
        insts = blk.instructions
        idx = 0
        while idx < len(insts):
            ins = insts[idx]
            si = ins.sync_info
            if si is not None and si.on_wait and len(si.on_wait) > 1:
                waits = list(si.on_wait)
                si.on_wait = [waits[-1]]
                for w in waits[:-1]:
                    nop = bass_rust.InstNoOp(
                        name=nc.get_next_instruction_name(), ins=[], outs=[]
                    )
                    nop.engine = ins.engine
                    nop.sync_info = mybir.SyncInfo(on_wait=[w], on_update=[])
                    nc.register_instruction(nop)
                    insts.insert(idx, nop)
                    idx += 1
                    n_split += 1
            idx += 1
    return n_split


def _build_units(species: np.ndarray):
    """Group atom indices by species into units of <=2 atoms."""
    units = []  # list of (species, [atom, ...])
    for s in range(S):
        atoms = [int(a) for a in np.nonzero(species == s)[0]]
        for i in range(0, len(atoms) - 1, 2):
            units.append((s, atoms[i : i + 2]))
        if len(atoms) % 2:
            units.append((s, atoms[-1:]))
    return units


# Weight blob column layout (one [128, WCOLS] f32 SBUF tile / DRAM tensor).
OFF_W0 = 0  # [s][c] at OFF_W0 + (s*NCH + c)*128, 128 cols, 128 parts
OFF_W1 = OFF_W0 + S * NCH * 128  # [s] at OFF_W1 + s*128, 128 cols
OFF_W2 = OFF_W1 + S * 128  # [s] at OFF_W2 + s*64, 64 cols
OFF_W3 = OFF_W2 + S * 64  # [s] at OFF_W3 + s, 1 col, 64 parts
OFF_B0 = OFF_W3 + S  # [s] at OFF_B0 + s, 1 col
OFF_B1 = OFF_B0 + S
OFF_B2 = OFF_B1 + S
WCOLS = OFF_B2 + S


def _pack_weights(W0, b0, W1, b1, W2, b2, W3, b3):
    c = math.sqrt(math.pi) / 2.0  # undo Derivative_Erf's 2/sqrt(pi)
    blob = np.zeros((128, WCOLS), np.float32)
    for s in range(S):
        for ch in range(NCH):
            blob[:, OFF_W0 + (s * NCH + ch) * 128 : OFF_W0 + (s * NCH + ch + 1) * 128] = (
                W0[s, ch * 128 : (ch + 1) * 128, :]
            )
        blob[:, OFF_W1 + s * 128 : OFF_W1 + (s + 1) * 128] = W1[s] * c
        blob[:, OFF_W2 + s * 64 : OFF_W2 + (s + 1) * 64] = W2[s] * c
        blob[:H3, OFF_W3 + s] = W3[s][:, 0] * c
        blob[:, OFF_B0 + s] = b0[s]
        blob[:, OFF_B1 + s] = b1[s]
        blob[:H3, OFF_B2 + s] = b2[s]
    return blob


def _pack_x(x, units):
    """Per-core flat x arrays.

    Per unit: block [128, NCH * w] where w = 256 * n_atoms; within chunk c the
    columns are (atom, b) so each layer-0 matmul rhs is [:, c*w:(c+1)*w].
    Returns (flat arrays per core, unit column offsets (in flat elems)).
    """
    # [A, D, B] so per (atom, chunk) the [128, BC] block is contiguous-ish
    xt = np.ascontiguousarray(x.transpose(1, 2, 0))  # [A, D, B]
    per_core = []
    offsets = []
    off = 0
    for s, atoms in units:
        offsets.append(off)
        off += 128 * NCH * 256 * len(atoms)
    total = off
    for core in range(NCORES):
        bsl = slice(core * BC, (core + 1) * BC)
        flat = np.empty(total, np.float32)
        for (s, atoms), uoff in zip(units, offsets):
            # [n_a, D, BC] -> [n_a, NCH, 128, BC] -> [NCH, 128, n_a, BC]
            blk = xt[atoms, :, bsl].reshape(len(atoms), NCH, 128, BC)
            blk = blk.transpose(2, 1, 0, 3)  # [128, NCH, n_a, BC]
            n = blk.size
            flat[uoff : uoff + n] = blk.reshape(-1)
        per_core.append(flat)
    return per_core, offsets, total


def _build_program(units, total_x, repeat=1):
    nc = bass.Bass()
    xin = nc.dram_tensor("xin", [total_x], F32, kind="ExternalInput")
    wts = nc.dram_tensor("wts", [128, WCOLS], F32, kind="ExternalInput")
    xout = nc.dram_tensor("xout", [1, 512], F32, kind="ExternalOutput")

    with tile.TileContext(nc) as tc:
        with (
            tc.tile_pool(name="wpool", bufs=1) as wpool,
            tc.tile_pool(name="xpool", bufs=4) as xpool,
            tc.tile_pool(name="apool", bufs=3) as apool,
            tc.tile_pool(name="opool", bufs=1) as opool,
            tc.tile_pool(name="ypool", bufs=2, space="PSUM") as ypool,
            tc.tile_pool(name="outp", bufs=1, space="PSUM") as outp,
        ):
            def body():
                wt = wpool.tile([128, WCOLS], F32, tag="wt")
                nc.sync.dma_start(wt[:], wts[:])

                out_ps = outp.tile([1, 512], F32, tag="ops")

                def w0_ap(s, ch):
                    o = OFF_W0 + (s * NCH + ch) * 128
                    return wt[:, o : o + 128]

                uoff = 0
                for ui, (s, atoms) in enumerate(units):
                    w = 256 * len(atoms)
                    xt_u = xpool.tile([128, NCH * 512], F32, tag="xu")
                    nc.sync.dma_start(
                        xt_u[:, : NCH * w],
                        xin[uoff : uoff + 128 * NCH * w].rearrange(
                            "(p n) -> p n", p=128
                        ),
                    )
                    uoff += 128 * NCH * w

                    # ---- layer 0: [384 -> 128] over d-chunks, N = w
                    y0 = ypool.tile([128, 512], F32, tag="y0")
                    for ch in range(NCH):
                        nc.tensor.matmul(
                            y0[:, :w],
                            w0_ap(s, ch),
                            xt_u[:, ch * w : (ch + 1) * w],
                            start=(ch == 0),
                            stop=(ch == NCH - 1),
                        )
                    a0 = apool.tile([128, 512], F32, tag="a0")
                    nc.scalar.activation(
                        a0[:, :w], y0[:, :w], AF.Derivative_Erf,
                        bias=wt[:, OFF_B0 + s : OFF_B0 + s + 1],
                    )

                    # ---- layer 1: [128 -> 128]
                    y1 = ypool.tile([128, 512], F32, tag="y1")
                    nc.tensor.matmul(
                        y1[:, :w],
                        wt[:, OFF_W1 + s * 128 : OFF_W1 + (s + 1) * 128],
                        a0[:, :w], start=True, stop=True,
                    )
                    a1 = apool.tile([128, 512], F32, tag="a1")
                    nc.scalar.activation(
                        a1[:, :w], y1[:, :w], AF.Derivative_Erf,
                        bias=wt[:, OFF_B1 + s : OFF_B1 + s + 1],
                    )

                    # ---- layer 2: [128 -> 64]
                    y2 = ypool.tile([64, 512], F32, tag="y2")
                    nc.tensor.matmul(
                        y2[:, :w],
                        wt[:, OFF_W2 + s * 64 : OFF_W2 + (s + 1) * 64],
                        a1[:, :w], start=True, stop=True,
                    )
                    a2 = apool.tile([64, 512], F32, tag="a2")
                    nc.scalar.activation(
                        a2[:, :w], y2[:, :w], AF.Derivative_Erf,
                        bias=wt[:H3, OFF_B2 + s : OFF_B2 + s + 1],
                    )

                    # ---- layer 3: [64 -> 1], accumulate over all units
                    nc.tensor.matmul(
                        out_ps[:, :w],
                        wt[:H3, OFF_W3 + s : OFF_W3 + s + 1],
                        a2[:, :w],
                        start=(ui == 0),
                        stop=(ui == len(units) - 1),
                    )

                ot = opool.tile([1, 512], F32, tag="ot")
                nc.vector.tensor_copy(ot[:], out_ps[:])
                nc.sync.dma_start(xout[:], ot[:])

            if repeat == 1:
                body()
            else:
                with tc.For_i(0, repeat, 1):
                    body()

    _split_multi_waits(nc)
    return nc


def _prep(x, species, W0, b0, W1, b1, W2, b2, W3, b3):
    x = np.asarray(x, np.float32)
    species = np.asarray(species)
    units = _build_units(species)
    blob = _pack_weights(
        np.asarray(W0, np.float32), np.asarray(b0, np.float32),
        np.asarray(W1, np.float32), np.asarray(b1, np.float32),
        np.asarray(W2, np.float32), np.asarray(b2, np.float32),
        np.asarray(W3, np.float32), np.asarray(b3, np.float32),
    )
    xs, _, total = _pack_x(x, units)
    b3sum = float(np.asarray(b3, np.float64)[species, 0].sum())
    in_maps = [{"xin": xs[c], "wts": blob} for c in range(NCORES)]
    return units, total, in_maps, b3sum


def kernel(x, species, W0, b0, W1, b1, W2, b2, W3, b3):
    global LAST_EXEC_NS, LAST_RESULTS
    units, total, in_maps, b3sum = _prep(
        x, species, W0, b0, W1, b1, W2, b2, W3, b3
    )
    nc = _build_program(units, total)
    res = run_bass_kernel_spmd(nc, in_maps, list(range(NCORES)))
    LAST_EXEC_NS = res.exec_time_ns
    LAST_RESULTS = res
    out = np.empty(B, np.float32)
    for c in range(NCORES):
        v = res.results[c]["xout"].reshape(512)
        out[c * BC : (c + 1) * BC] = (
            v[:256].astype(np.float64) + v[256:].astype(np.float64) + b3sum
        ).astype(np.float32)
    return out


def bench(x, species, W0, b0, W1, b1, W2, b2, W3, b3,
          reps=(256, 16384), tries=3):
    """Per-invocation HW time via on-device For_i loop slope.

    Runs the kernel body R times inside one NEFF for each R in reps and
    wall-clocks the execute call; the slope between the two R values
    cancels tunnel/upload overhead.  Includes ~2-3us/iter of Tile loop
    back-edge barrier cost (constant across kernel versions).
    """
    import time as _time

    units, total, in_maps, _ = _prep(
        x, species, W0, b0, W1, b1, W2, b2, W3, b3
    )
    cores = list(range(NCORES))
    timings = {}
    for R in reps:
        nc = _build_program(units, total, repeat=R)
        ts = []
        for _ in range(tries):
            t0 = _time.perf_counter()
            run_bass_kernel_spmd(nc, in_maps, cores)
            ts.append(_time.perf_counter() - t0)
        timings[R] = min(ts[1:]) if len(ts) > 1 else ts[0]
    r0, r1 = min(reps), max(reps)
    ns = (timings[r1] - timings[r0]) / (r1 - r0) * 1e9
    return ns, timings


# revision 9
# speedup vs baseline: 2.6058x; 2.6058x over previous
"""Trainium2 Bass kernel for the per-species (MoE-routed) atom MLP net.

Computation (see reference):
  x: [B=2048, A=64, D=384] f32, species: [A] int32 in [0, S=4)
  4-layer per-species MLP 384->128->128->64->1 with gaussian act exp(-y^2)
  between layers, then sum over atoms -> out [B].

Strategy:
  - Data-parallel over B across 8 cores (B_c = 256), no collectives.
  - Host-side: repack x into [feature-on-partition, batch-on-free] layout so
    all device DMAs are fully contiguous; group atoms by species into "units"
    of up to 2 atoms (N = 256 * n_atoms <= 512 matmul free dim, fp32).
  - exp(-(y+b)^2) is computed in ONE ScalarE op via Derivative_Erf:
    d/dx erf(x) = (2/sqrt(pi)) * exp(-x^2); the 2/sqrt(pi) factor is folded
    into the next layer's weights on the host (sqrt(pi)/2 scaling).
  - Layer 3 matmuls accumulate all units into one [1, 512] PSUM bank; the
    final fold (cols b + cols 256+b, + sum of b3 biases) happens on host.
"""

import math

import numpy as np

import concourse.bass as bass
import concourse.mybir as mybir
import concourse.tile as tile
from concourse.bass_utils import run_bass_kernel_spmd
from concourse.vector_clock import ScopedClock

AF = mybir.ActivationFunctionType
F32 = mybir.dt.float32
F32R = mybir.dt.float32r

# Matmul input dtype: float32r streams 1 col/cycle on the PE (vs 4 for
# float32) at ~TF32 effective multiply precision, fp32 accumulation.
USE_F32R = True

B, A, D, S = 2048, 64, 384, 4
H1, H2, H3 = 128, 128, 64
NCORES = 8
BC = B // NCORES  # 256 batch per core
NCH = D // 128  # 3 k-chunks for layer 0

# Set by test harness to collect a profile; kernel() stores exec_time_ns here.
PROFILE = False
LAST_EXEC_NS = None
LAST_RESULTS = None

# ---------------------------------------------------------------------------
# Walrus in this toolchain rejects >1 sync-wait per instruction ("Too many
# sync wait commands", CoreV3GenImpl setupSyncWait).  Tile's semaphore
# assignment freely attaches several waits to one instruction, so any real
# Tile kernel trips it.  Post-pass: hoist all but one wait onto injected
# NoOps on the same engine queue immediately before the instruction — the
# queue executes them in order, so the blocking semantics are identical.
# ---------------------------------------------------------------------------


def _split_multi_waits(nc):
    import bass_rust

    n_split = 0
    for blk in nc.main_func.blocks:
        insts = blk.instructions
        idx = 0
        while idx < len(insts):
            ins = insts[idx]
            si = ins.sync_info
            if si is not None and si.on_wait and len(si.on_wait) > 1:
                waits = list(si.on_wait)
                si.on_wait = [waits[-1]]
                for w in waits[:-1]:
                    nop = bass_rust.InstNoOp(
                        name=nc.get_next_instruction_name(), ins=[], outs=[]
                    )
                    nop.engine = ins.engine
                    nop.sync_info = mybir.SyncInfo(on_wait=[w], on_update=[])
                    nc.register_instruction(nop)
                    insts.insert(idx, nop)
                    idx += 1
                    n_split += 1
            idx += 1
    return n_split


def _build_units(species: np.ndarray):
    """Group atom indices by species into units of <=2 atoms."""
    units = []  # list of (species, [atom, ...])
    for s in range(S):
        atoms = [int(a) for a in np.nonzero(species == s)[0]]
        for i in range(0, len(atoms) - 1, 2):
            units.append((s, atoms[i : i + 2]))
        if len(atoms) % 2:
            units.append((s, atoms[-1:]))
    return units


# Weight blob column layout (one [128, WCOLS] f32 SBUF tile / DRAM tensor).
OFF_W0 = 0  # [s][c] at OFF_W0 + (s*NCH + c)*128, 128 cols, 128 parts
OFF_W1 = OFF_W0 + S * NCH * 128  # [s] at OFF_W1 + s*128, 128 cols
OFF_W2 = OFF_W1 + S * 128  # [s] at OFF_W2 + s*64, 64 cols
OFF_W3 = OFF_W2 + S * 64  # [s] at OFF_W3 + s, 1 col, 64 parts
OFF_B0 = OFF_W3 + S  # [s] at OFF_B0 + s, 1 col
OFF_B1 = OFF_B0 + S
OFF_B2 = OFF_B1 + S
WCOLS = OFF_B2 + S


def _pack_weights(W0, b0, W1, b1, W2, b2, W3, b3):
    c = math.sqrt(math.pi) / 2.0  # undo Derivative_Erf's 2/sqrt(pi)
    blob = np.zeros((128, WCOLS), np.float32)
    for s in range(S):
        for ch in range(NCH):
            blob[:, OFF_W0 + (s * NCH + ch) * 128 : OFF_W0 + (s * NCH + ch + 1) * 128] = (
                W0[s, ch * 128 : (ch + 1) * 128, :]
            )
        blob[:, OFF_W1 + s * 128 : OFF_W1 + (s + 1) * 128] = W1[s] * c
        blob[:, OFF_W2 + s * 64 : OFF_W2 + (s + 1) * 64] = W2[s] * c
        blob[:H3, OFF_W3 + s] = W3[s][:, 0] * c
        blob[:, OFF_B0 + s] = b0[s]
        blob[:, OFF_B1 + s] = b1[s]
        blob[:H3, OFF_B2 + s] = b2[s]
    return blob


def _pack_x(x, units):
    """Per-core flat x arrays.

    Per unit: block [128, NCH * w] where w = 256 * n_atoms; within chunk c the
    columns are (atom, b) so each layer-0 matmul rhs is [:, c*w:(c+1)*w].
    Returns (flat arrays per core, unit column offsets (in flat elems)).
    """
    # [A, D, B] so per (atom, chunk) the [128, BC] block is contiguous-ish
    xt = np.ascontiguousarray(x.transpose(1, 2, 0))  # [A, D, B]
    per_core = []
    offsets = []
    off = 0
    for s, atoms in units:
        offsets.append(off)
        off += 128 * NCH * 256 * len(atoms)
    total = off
    for core in range(NCORES):
        bsl = slice(core * BC, (core + 1) * BC)
        flat = np.empty(total, np.float32)
        for (s, atoms), uoff in zip(units, offsets):
            # [n_a, D, BC] -> [n_a, NCH, 128, BC] -> [NCH, 128, n_a, BC]
            blk = xt[atoms, :, bsl].reshape(len(atoms), NCH, 128, BC)
            blk = blk.transpose(2, 1, 0, 3)  # [128, NCH, n_a, BC]
            n = blk.size
            flat[uoff : uoff + n] = blk.reshape(-1)
        per_core.append(flat)
    return per_core, offsets, total


def _build_program(units, total_x, repeat=1):
    MDT = F32R if USE_F32R else F32
    nc = bass.Bass()
    xin = nc.dram_tensor("xin", [total_x], MDT, kind="ExternalInput")
    wts = nc.dram_tensor("wts", [128, WCOLS], MDT, kind="ExternalInput")
    xout = nc.dram_tensor("xout", [1, 512], F32, kind="ExternalOutput")

    with tile.TileContext(nc) as tc:
        with (
            tc.tile_pool(name="wpool", bufs=1) as wpool,
            tc.tile_pool(name="xpool", bufs=4) as xpool,
            tc.tile_pool(name="apool", bufs=3) as apool,
            tc.tile_pool(name="opool", bufs=1) as opool,
            tc.tile_pool(name="ypool", bufs=2, space="PSUM") as ypool,
            tc.tile_pool(name="outp", bufs=1, space="PSUM") as outp,
        ):
            def body():
                wt = wpool.tile([128, WCOLS], MDT, tag="wt")
                nc.sync.dma_start(wt[:], wts[:])

                out_ps = outp.tile([1, 512], F32, tag="ops")

                def w0_ap(s, ch):
                    o = OFF_W0 + (s * NCH + ch) * 128
                    return wt[:, o : o + 128]

                def bias_ap(off, s, p=128):
                    ap = wt[:p, off + s : off + s + 1]
                    return ap.bitcast(F32) if MDT is F32R else ap

                uoff = 0
                for ui, (s, atoms) in enumerate(units):
                    w = 256 * len(atoms)
                    xt_u = xpool.tile([128, NCH * 512], MDT, tag="xu")
                    nc.sync.dma_start(
                        xt_u[:, : NCH * w],
                        xin[uoff : uoff + 128 * NCH * w].rearrange(
                            "(p n) -> p n", p=128
                        ),
                    )
                    uoff += 128 * NCH * w

                    # ---- layer 0: [384 -> 128] over d-chunks, N = w
                    y0 = ypool.tile([128, 512], F32, tag="y0")
                    for ch in range(NCH):
                        nc.tensor.matmul(
                            y0[:, :w],
                            w0_ap(s, ch),
                            xt_u[:, ch * w : (ch + 1) * w],
                            start=(ch == 0),
                            stop=(ch == NCH - 1),
                        )
                    a0 = apool.tile([128, 512], MDT, tag="a0")
                    nc.scalar.activation(
                        a0[:, :w], y0[:, :w], AF.Derivative_Erf,
                        bias=bias_ap(OFF_B0, s),
                    )

                    # ---- layer 1: [128 -> 128]
                    y1 = ypool.tile([128, 512], F32, tag="y1")
                    nc.tensor.matmul(
                        y1[:, :w],
                        wt[:, OFF_W1 + s * 128 : OFF_W1 + (s + 1) * 128],
                        a0[:, :w], start=True, stop=True,
                    )
                    a1 = apool.tile([128, 512], MDT, tag="a1")
                    nc.scalar.activation(
                        a1[:, :w], y1[:, :w], AF.Derivative_Erf,
                        bias=bias_ap(OFF_B1, s),
                    )

                    # ---- layer 2: [128 -> 64]
                    y2 = ypool.tile([64, 512], F32, tag="y2")
                    nc.tensor.matmul(
                        y2[:, :w],
                        wt[:, OFF_W2 + s * 64 : OFF_W2 + (s + 1) * 64],
                        a1[:, :w], start=True, stop=True,
                    )
                    a2 = apool.tile([64, 512], MDT, tag="a2")
                    nc.scalar.activation(
                        a2[:, :w], y2[:, :w], AF.Derivative_Erf,
                        bias=bias_ap(OFF_B2, s, p=H3),
                    )

                    # ---- layer 3: [64 -> 1], accumulate over all units
                    nc.tensor.matmul(
                        out_ps[:, :w],
                        wt[:H3, OFF_W3 + s : OFF_W3 + s + 1],
                        a2[:, :w],
                        start=(ui == 0),
                        stop=(ui == len(units) - 1),
                    )

                ot = opool.tile([1, 512], F32, tag="ot")
                nc.vector.tensor_copy(ot[:], out_ps[:])
                nc.sync.dma_start(xout[:], ot[:])

            if repeat == 1:
                body()
            else:
                with tc.For_i(0, repeat, 1):
                    body()

    _split_multi_waits(nc)
    return nc


def _prep(x, species, W0, b0, W1, b1, W2, b2, W3, b3):
    x = np.asarray(x, np.float32)
    species = np.asarray(species)
    units = _build_units(species)
    blob = _pack_weights(
        np.asarray(W0, np.float32), np.asarray(b0, np.float32),
        np.asarray(W1, np.float32), np.asarray(b1, np.float32),
        np.asarray(W2, np.float32), np.asarray(b2, np.float32),
        np.asarray(W3, np.float32), np.asarray(b3, np.float32),
    )
    xs, _, total = _pack_x(x, units)
    b3sum = float(np.asarray(b3, np.float64)[species, 0].sum())
    in_maps = [{"xin": xs[c], "wts": blob} for c in range(NCORES)]
    return units, total, in_maps, b3sum


def kernel(x, species, W0, b0, W1, b1, W2, b2, W3, b3):
    global LAST_EXEC_NS, LAST_RESULTS
    units, total, in_maps, b3sum = _prep(
        x, species, W0, b0, W1, b1, W2, b2, W3, b3
    )
    nc = _build_program(units, total)
    res = run_bass_kernel_spmd(nc, in_maps, list(range(NCORES)))
    LAST_EXEC_NS = res.exec_time_ns
    LAST_RESULTS = res
    out = np.empty(B, np.float32)
    for c in range(NCORES):
        v = res.results[c]["xout"].reshape(512)
        out[c * BC : (c + 1) * BC] = (
            v[:256].astype(np.float64) + v[256:].astype(np.float64) + b3sum
        ).astype(np.float32)
    return out


def bench(x, species, W0, b0, W1, b1, W2, b2, W3, b3,
          reps=(256, 16384), tries=3):
    """Per-invocation HW time via on-device For_i loop slope.

    Runs the kernel body R times inside one NEFF for each R in reps and
    wall-clocks the execute call; the slope between the two R values
    cancels tunnel/upload overhead.  Includes ~2-3us/iter of Tile loop
    back-edge barrier cost (constant across kernel versions).
    """
    import time as _time

    units, total, in_maps, _ = _prep(
        x, species, W0, b0, W1, b1, W2, b2, W3, b3
    )
    cores = list(range(NCORES))
    timings = {}
    for R in reps:
        nc = _build_program(units, total, repeat=R)
        ts = []
        for _ in range(tries):
            t0 = _time.perf_counter()
            run_bass_kernel_spmd(nc, in_maps, cores)
            ts.append(_time.perf_counter() - t0)
        timings[R] = min(ts[1:]) if len(ts) > 1 else ts[0]
    r0, r1 = min(reps), max(reps)
    ns = (timings[r1] - timings[r0]) / (r1 - r0) * 1e9
    return ns, timings
